# revision 48
# baseline (speedup 1.0000x reference)
"""Distributed GQA attention (RoPE, additive mask) for one TRN2 chip (8 NeuronCores).

Reference semantics (B=4, S=2048, HID=576, 9 q-heads / 3 kv-heads, HD=64):
    q = (x @ wq.T) -> rope;  k = (x @ wk.T) -> rope;  v = x @ wv.T
    scores = q k^T / sqrt(HD) + mask;  attn = softmax(scores);  out = (attn v) @ wo.T

Sharding: 8 cores = 4 batches x 2 query-shards. Each query shard owns 8 of the
16 query tiles (128 tokens each), paired {i, 15-i} alternately so causal work
is balanced. Each core computes all 9 heads for its queries (k/v projections
over the full sequence are duplicated between the 2 cores of a batch --
cheaper than any collective). No cross-core communication.

Device layouts are transposed ([feature, token]) so the TensorEngine contracts
over the head dim; the host pre-transposes inputs, gathers RoPE tables at
position_ids, folds 1/sqrt(HD) into wq, and casts to bf16. Softmax skips the
max-subtraction (scores are O(1) by construction); the denominator comes from
an appended ones-column in V. Masking multiplies exp(scores) by exp(mask);
when the mask is exactly causal this reduces to skipping upper-triangle tiles
plus one per-core 128x128 block multiply per k-tile (SPMD runs one graph on
both shards, so the block mask -- triangular / zeros / ones -- is a per-core
input).
"""

import os
import numpy as np
import ml_dtypes

B, S, HID = 4, 2048, 576
NH, NKV, HD = 9, 3, 64
GQ = NH // NKV
KV = NKV * HD          # 192
PT = 128               # token tile (partition dim)
NQT = S // PT          # 16 query tiles per batch
SHARDS = ([0, 2, 4, 6, 9, 11, 13, 15], [1, 3, 5, 7, 8, 10, 12, 14])
NSH = 8                # q-tiles per shard
QL = NSH * PT          # 1024 queries per core
N_CORES = 8
BF16 = ml_dtypes.bfloat16

# hidden-dim partition chunks: 576 = 4*128 + 64
H_CH = [(0, 128), (128, 128), (256, 128), (384, 128), (512, 64)]

LAST_EXEC_NS = None
LAST_RESULTS = None
_NC_CACHE = {}


# ----------------------------------------------------------------- helpers

def _union_count(kt):
    """max over shards of #tiles >= kt (SPMD union coverage)."""
    return max(sum(1 for t in q if t >= kt) for q in SHARDS)


def _sstart(mode, kt):
    """packed-column start of the union suffix for k-tile kt."""
    if mode != "causal":
        return 0
    return (NSH - _union_count(kt)) * PT


def _chunks(a, b, step=512):
    """split [a, b) at multiples of `step`"""
    out = []
    while a < b:
        c = min(b, (a // step + 1) * step)
        out.append((a, c))
        a = c
    return out


def _mask_mode(attention_mask):
    m = attention_mask[:, 0]  # [B, S, S]
    if not np.any(m):
        return "zeros"
    ki = np.arange(S)
    keep = ki[None, :] <= ki[:, None]  # [S(q), S(k)] lower triangle inclusive
    if np.all(m[:, keep] == 0) and np.all(m[:, ~keep] <= -1e4):
        return "causal"
    return "general"


# ----------------------------------------------------------------- host prep

def _dup_cos(c):
    # c: [T, 64] fp32 -> [128, T] bf16, row r -> cos[dim r%64]
    ct = np.ascontiguousarray(c.T)
    return np.concatenate([ct, ct], axis=0).astype(BF16)


def _dup_sin_signed(s):
    # s: [T, 64] fp32 -> [128, T] bf16, row r: -sin[r%64] if r%64<32 else +sin
    st = s.T.copy()
    st[0:32] = -st[0:32]
    return np.concatenate([st, st], axis=0).astype(BF16)


def _prep_inputs(x, cos, sin, position_ids, attention_mask, wq, wk, wv, wo, mode):
    pos = np.asarray(position_ids)
    wqt = np.ascontiguousarray(wq.T / np.sqrt(HD)).astype(BF16)
    wkt = np.ascontiguousarray(wk.T).astype(BF16)
    wvt = np.ascontiguousarray(wv.T).astype(BF16)
    wot = np.ascontiguousarray(wo.T).astype(BF16)
    triu = np.triu(np.ones((PT, PT), np.float32))

    in_maps = []
    for c in range(N_CORES):
        b, sh = divmod(c, 2)
        qtiles = SHARDS[sh]
        qidx = np.concatenate([np.arange(t * PT, (t + 1) * PT) for t in qtiles])
        cosb = cos[pos[b]]  # [S, 64]
        sinb = sin[pos[b]]
        m = {
            "xt": np.ascontiguousarray(x[b].T).astype(BF16),
            "xq": np.ascontiguousarray(x[b][qidx].T).astype(BF16),
            "wqt": wqt, "wkt": wkt, "wvt": wvt, "wot": wot,
            "cosq": _dup_cos(cosb[qidx]), "ssq": _dup_sin_signed(sinb[qidx]),
            "cosk": _dup_cos(cosb), "ssk": _dup_sin_signed(sinb),
        }
        if mode == "causal":
            # per-core block mask for the first union-suffix block of each kt:
            # diagonal (triangular) if this core owns kt at that position,
            # zeros if its tile there is < kt (fully masked), ones otherwise.
            dm = np.ones((NQT, PT, PT), np.float32)
            for kt in range(NQT):
                p = NSH - _union_count(kt)
                t = qtiles[p]
                if t == kt:
                    dm[kt] = triu
                elif t < kt:
                    dm[kt] = 0.0
            m["dmask"] = dm.reshape(NQT * PT, PT).astype(BF16)
        elif mode == "general":
            mb = attention_mask[b, 0][qidx]  # [QL, S] additive mask, our queries
            m["emskT"] = np.exp(np.minimum(mb.T, 30.0)).astype(BF16)  # [S, QL]
        in_maps.append(m)
    return in_maps


# ------------------------------------------------------------- device kernel

def _head_rows(tiles, h):
    """64-row slice of a row-partitioned [R,C] matrix (tiles of <=128 rows)."""
    off = h * HD
    return tiles[off // PT][off % PT:off % PT + HD]


def _build(mode):
    import concourse.mybir as mybir
    from concourse import bacc, tile
    from contextlib import ExitStack

    dt = mybir.dt
    f32, bf = dt.float32, dt.bfloat16
    AF = mybir.ActivationFunctionType

    nc = bacc.Bacc(
        "TRN2", target_bir_lowering=False, debug=False,
        enable_asserts=False, num_devices=N_CORES,
    )

    def din(name, shape, dty=bf):
        return nc.dram_tensor(name, shape, dty, kind="ExternalInput").ap()

    xt_d = din("xt", [HID, S])
    xq_d = din("xq", [HID, QL])
    wqt_d = din("wqt", [HID, HID])
    wkt_d = din("wkt", [HID, KV])
    wvt_d = din("wvt", [HID, KV])
    wot_d = din("wot", [HID, HID])
    cosq_d = din("cosq", [PT, QL])
    ssq_d = din("ssq", [PT, QL])
    cosk_d = din("cosk", [PT, S])
    ssk_d = din("ssk", [PT, S])
    if mode == "causal":
        dmask_d = din("dmask", [NQT * PT, PT])
    elif mode == "general":
        emskT_d = din("emskT", [S, QL])
    out_d = nc.dram_tensor("out", [HID, QL], f32, kind="ExternalOutput").ap()

    with tile.TileContext(nc) as tc, ExitStack() as ctx:
        ctx.enter_context(nc.allow_low_precision(reason="bf16 attention kernel"))
        sb = ctx.enter_context(tc.tile_pool(name="persist", bufs=1))
        rtmp = ctx.enter_context(tc.tile_pool(name="rtmp", bufs=3))
        attnp = ctx.enter_context(tc.tile_pool(name="attnp", bufs=8))
        outp = ctx.enter_context(tc.tile_pool(name="outp", bufs=3))
        psum = ctx.enter_context(tc.tile_pool(name="psum", bufs=1, space="PSUM"))
        # DRAM bounce for the per-head denominator broadcast (partition-step-0
        # APs are rejected for SBUF sources but fine for DRAM sources)
        dramp = ctx.enter_context(tc.tile_pool(name="dramp", bufs=2, space="DRAM"))

        # ---- persistent loads (row-chunked by 128) ----
        def load_rows(dram, rows, cols, tag):
            tiles = []
            off = 0
            i = 0
            while off < rows:
                sz = min(PT, rows - off)
                t = sb.tile([sz, cols], bf, tag=f"{tag}{i}", name=f"{tag}{i}")
                nc.sync.dma_start(out=t[:, :], in_=dram[off:off + sz, :])
                tiles.append(t)
                off += sz
                i += 1
            return tiles

        def load_rows_cols(dram, rows, cols, tag, cstep=512):
            """like load_rows but DMA in column chunks so consumers of early
            columns unblock before the whole tensor lands"""
            tiles = []
            bounds = []
            off = 0
            i = 0
            while off < rows:
                sz = min(PT, rows - off)
                t = sb.tile([sz, cols], bf, tag=f"{tag}{i}", name=f"{tag}{i}")
                tiles.append(t)
                bounds.append((off, sz))
                off += sz
                i += 1
            for c0, c1 in _chunks(0, cols, cstep):
                for t, (off, sz) in zip(tiles, bounds):
                    nc.sync.dma_start(
                        out=t[:, c0:c1], in_=dram[off:off + sz, c0:c1])
            return tiles

        # load order matters: k/v-projection inputs first
        wkt_sb = load_rows(wkt_d, HID, KV, "wkt")
        wvt_sb = load_rows(wvt_d, HID, KV, "wvt")
        cosk_sb = load_rows(cosk_d, PT, S, "cosk")[0]
        ssk_sb = load_rows(ssk_d, PT, S, "ssk")[0]
        xt_sb = load_rows_cols(xt_d, HID, S, "xt")
        wqt_sb = load_rows(wqt_d, HID, HID, "wqt")
        cosq_sb = load_rows(cosq_d, PT, QL, "cosq")[0]
        ssq_sb = load_rows(ssq_d, PT, QL, "ssq")[0]
        xq_sb = load_rows_cols(xq_d, HID, QL, "xq")
        if mode == "causal":
            dmask_sb = load_rows(dmask_d, NQT * PT, PT, "dmask")
        elif mode == "general":
            emskT_sb = load_rows(emskT_d, S, QL, "emskT")
        wot_sb = load_rows(wot_d, HID, HID, "wot")



        # ---- projections + RoPE ----
        def rope(ps, sz, cos_sb, ss_sb, dsts, c0, c1):
            """dsts: per-head [64, *] sbuf tiles for the sz//64 heads in ps.

            q' = q*cos + rot(q)*sin, with rot/sign folded into partner-row
            reads (psum may be partition-shifted vs the sbuf operands).
            """
            w = c1 - c0
            t1 = rtmp.tile([PT, 512], bf, tag="rt1", name="rt1")
            nc.vector.tensor_mul(t1[:sz, :w], ps[:sz, :w], cos_sb[:sz, c0:c1])
            t2 = rtmp.tile([PT, 512], bf, tag="rt2", name="rt2")
            for g in range(sz // 32):
                p = g ^ 1
                nc.vector.tensor_mul(
                    t2[g * 32:(g + 1) * 32, :w],
                    ps[p * 32:p * 32 + 32, :w],
                    ss_sb[g * 32:g * 32 + 32, c0:c1],
                )
            for hh, dst in enumerate(dsts):
                nc.vector.tensor_add(
                    dst[0:HD, c0:c1],
                    t1[hh * HD:(hh + 1) * HD, :w],
                    t2[hh * HD:(hh + 1) * HD, :w],
                )

        qp_sb = [sb.tile([HD, QL], bf, tag=f"qp{h}", name=f"qp{h}")
                 for h in range(NH)]
        kp_sb = [sb.tile([HD, S], bf, tag=f"kp{g}", name=f"kp{g}")
                 for g in range(NKV)]
        vext_sb = [sb.tile([PT, NKV * (HD + 1)], bf, tag=f"vext{kt}",
                           name=f"vext{kt}") for kt in range(NQT)]
        ao_sb = [sb.tile([sz, QL], bf, tag=f"ao{i}", name=f"ao{i}")
                 for i, (_, sz) in enumerate(H_CH)]

        def emit_qproj(i):
            off, sz = H_CH[i]
            for c0, c1 in _chunks(0, QL):
                ps = psum.tile([PT, 512], f32, tag="ps", bufs=6, name="ps")
                for j, (hoff, hsz) in enumerate(H_CH):
                    nc.tensor.matmul(
                        ps[:sz, :c1 - c0],
                        lhsT=wqt_sb[j][:, off:off + sz],
                        rhs=xq_sb[j][:, c0:c1],
                        start=(j == 0), stop=(j == len(H_CH) - 1),
                    )
                rope(ps, sz, cosq_sb, ssq_sb,
                     qp_sb[2 * i:2 * i + sz // HD], c0, c1)

        def emit_kproj(i, c0, c1):
            off, sz = [(0, 128), (128, 64)][i]
            ps = psum.tile([PT, 512], f32, tag="ps", bufs=6, name="ps")
            for j, (hoff, hsz) in enumerate(H_CH):
                nc.tensor.matmul(
                    ps[:sz, :c1 - c0],
                    lhsT=wkt_sb[j][:, off:off + sz],
                    rhs=xt_sb[j][:, c0:c1],
                    start=(j == 0), stop=(j == len(H_CH) - 1),
                )
            rope(ps, sz, cosk_sb, ssk_sb, kp_sb[2 * i:2 * i + sz // HD], c0, c1)

        def emit_vproj(kt):
            # v with appended ones column: vext[kt] is [128, 3*65]
            nc.gpsimd.memset(vext_sb[kt][:, :], 1.0)
            ps = psum.tile([PT, KV], f32, tag="ps", bufs=6, name="ps")
            for j, (hoff, hsz) in enumerate(H_CH):
                nc.tensor.matmul(
                    ps[:, :],
                    lhsT=xt_sb[j][:, kt * PT:(kt + 1) * PT],
                    rhs=wvt_sb[j][:, :],
                    start=(j == 0), stop=(j == len(H_CH) - 1),
                )
            src = ps[:, :].rearrange("p (g c) -> p g c", c=HD)
            dst = vext_sb[kt][:, :].rearrange(
                "p (g c) -> p g c", c=HD + 1)[:, :, 0:HD]
            nc.vector.tensor_copy(dst, src)

        kts = [kt for kt in range(NQT) if _sstart(mode, kt) < QL]
        last_kt_bank = {
            c0: max(kt for kt in kts if _sstart(mode, kt) < c0 + 512)
            for c0 in range(0, QL, 512)
        }

        def emit_head(h):
            g = h // GQ
            q_ap = qp_sb[h]
            k_ap = kp_sb[g]
            num = psum.tile([HD + 1, QL], f32, tag="num", bufs=1, name="num")
            for kt in kts:
                s0 = _sstart(mode, kt)
                at = attnp.tile([PT, QL], bf, tag="attn", name="attn")
                for c0, c1 in reversed(_chunks(s0, QL)):
                    sc = psum.tile([PT, 512], f32, tag="ps", bufs=6, name="sc")
                    nc.tensor.matmul(
                        sc[:, :c1 - c0],
                        lhsT=k_ap[:, kt * PT:(kt + 1) * PT],
                        rhs=q_ap[:, c0:c1],
                        start=True, stop=True,
                    )
                    nc.scalar.activation(at[:, c0:c1], sc[:, :c1 - c0], AF.Exp)
                vk = vext_sb[kt][:, g * (HD + 1):(g + 1) * (HD + 1)]
                chunks = _chunks(s0, QL)

                def av(c0, c1):
                    nc.tensor.matmul(
                        num[:, c0:c1], lhsT=vk, rhs=at[:, c0:c1],
                        start=(kt == kts[0]),
                        stop=(kt == last_kt_bank[(c0 // 512) * 512]),
                        skip_group_check=True,
                    )

                if mode == "general":
                    nc.vector.tensor_mul(at[:, :], at[:, :], emskT_sb[kt][:, :])
                    for c0, c1 in chunks:
                        av(c0, c1)
                else:
                    # later chunks don't touch the masked block: issue their
                    # AV matmuls before the mask multiply lands
                    for c0, c1 in chunks[1:]:
                        av(c0, c1)
                    if mode == "causal":
                        nc.vector.tensor_mul(
                            at[:, s0:s0 + PT], at[:, s0:s0 + PT],
                            dmask_sb[kt][:, :])
                    av(*chunks[0])
            # copy num to SBUF immediately so the single psum slot frees for
            # the next head; normalize from the copy. The denominator row is
            # copied to a base-partition-0 tile (PSUM sources may shift
            # partitions; SBUF ones cannot) because the custom-DVE
            # reciprocal_approx_fast only works at base partition 0.
            nsb = rtmp.tile([HD, QL], f32, tag="nsb", bufs=2, name="nsb")
            for c0, c1 in _chunks(0, QL):
                nc.vector.tensor_copy(nsb[:, c0:c1], num[0:HD, c0:c1])
            den = rtmp.tile([1, QL], f32, tag="den", bufs=2, name="den")
            nc.vector.tensor_copy(den[:, :], num[HD:HD + 1, :])
            rec = rtmp.tile([1, QL], f32, tag="rec", bufs=2, name="rec")
            nc.vector.reciprocal_approx_fast(rec[:, :], den[:, :])
            rscr = dramp.tile([1, QL], f32, tag="rscr", name="rscr")
            nc.sync.dma_start(out=rscr[:, :], in_=rec[:, :])
            reps = rtmp.tile([HD, QL], f32, tag="reps", bufs=2, name="reps")
            nc.sync.dma_start(
                out=reps[:, :], in_=rscr[0:1, :].broadcast_to([HD, QL]))
            ao = _head_rows(ao_sb, h)
            for c0, c1 in _chunks(0, QL):
                nc.vector.tensor_mul(
                    ao[:, c0:c1], nsb[:, c0:c1], reps[:, c0:c1])

        # ---- output projection: outT[o, t] = sum_f wot[f, o] * ao[f, t] ----
        # j = 0..3 only need heads 0..7, so a few groups can accumulate while
        # head 8 is still in flight; j = 4 (head 8) lands afterwards
        NJ = len(H_CH)

        def emit_oproj_head(i, c0, c1):
            off, sz = H_CH[i]
            ps = psum.tile([PT, 512], f32, tag="ps", bufs=6, name="ps")
            for j in range(NJ - 1):
                nc.tensor.matmul(
                    ps[:sz, :],
                    lhsT=wot_sb[j][:, off:off + sz],
                    rhs=ao_sb[j][:, c0:c1],
                    start=(j == 0), stop=False,
                    skip_group_check=True,
                )
            return ps

        def emit_oproj_tail(i, c0, c1, ps):
            off, sz = H_CH[i]
            nc.tensor.matmul(
                ps[:sz, :],
                lhsT=wot_sb[NJ - 1][:, off:off + sz],
                rhs=ao_sb[NJ - 1][:, c0:c1],
                start=False, stop=True,
                skip_group_check=True,
            )
            ot = outp.tile([PT, 512], f32, tag="ot", name="ot")
            nc.scalar.copy(ot[:sz, :], ps[:sz, :])
            nc.sync.dma_start(out=out_d[off:off + sz, c0:c1], in_=ot[:sz, :])

        ALL_GROUPS = [(i, c0, c1) for i in range(NJ)
                      for c0, c1 in _chunks(0, QL)]

        # ---- emission schedule ----
        # k/v projections first (v matmuls fill the PE while k's RoPE runs on
        # the vector engine); then q chunks just-in-time interleaved with
        # attention heads so later RoPE overlaps earlier heads' PE work.
        # kv-heads 0/1 (tile row 0) first: heads 0..5 need only those; kv-head
        # 2's projection overlaps the early heads' attention
        for n, (c0, c1) in enumerate(_chunks(0, S)):
            emit_kproj(0, c0, c1)
            for kt in range(4 * n, 4 * n + 4):
                emit_vproj(kt)
        emit_qproj(0)
        emit_qproj(1)
        oproj_ps = {}
        for h in range(NH):
            if h == 1:
                emit_kproj(1, 0, 512)
                emit_kproj(1, 512, 1024)
            if h == 2:
                emit_kproj(1, 1024, 1536)
                emit_kproj(1, 1536, 2048)
            if h == NH - 1:
                # j=0..3 accumulation for the first group overlaps head 8
                for g in ALL_GROUPS[:1]:
                    oproj_ps[g] = emit_oproj_head(*g)
            emit_head(h)
            if 2 + h < len(H_CH):
                emit_qproj(2 + h)
        for g in ALL_GROUPS[1:]:
            oproj_ps[g] = emit_oproj_head(*g)

        for g in ALL_GROUPS:
            emit_oproj_tail(*g, oproj_ps[g])

    nc.compile()
    return nc


def _get_nc(mode):
    if mode not in _NC_CACHE:
        _NC_CACHE[mode] = _build(mode)
    return _NC_CACHE[mode]


def kernel(x, cos, sin, position_ids, attention_mask, wq, wk, wv, wo):
    global LAST_EXEC_NS
    from concourse.bass_utils import run_bass_kernel_spmd

    x = np.asarray(x, np.float32)
    cos = np.asarray(cos, np.float32)
    sin = np.asarray(sin, np.float32)
    position_ids = np.asarray(position_ids)
    attention_mask = np.asarray(attention_mask, np.float32)
    mode = _mask_mode(attention_mask)

    in_maps = _prep_inputs(
        x, cos, sin, position_ids, attention_mask, wq, wk, wv, wo, mode)
    nc = _get_nc(mode)

    trace = os.environ.get("KERNEL_TRACE", "1") != "0"
    try:
        res = run_bass_kernel_spmd(
            nc, in_maps, core_ids=list(range(N_CORES)), trace=trace)
    except Exception:
        if not trace:
            raise
        res = run_bass_kernel_spmd(
            nc, in_maps, core_ids=list(range(N_CORES)), trace=False)
    LAST_EXEC_NS = res.exec_time_ns
    globals()["LAST_RESULTS"] = res

    y = np.empty((B, S, NH * HD), np.float32)
    for c in range(N_CORES):
        b, sh = divmod(c, 2)
        qidx = np.concatenate(
            [np.arange(t * PT, (t + 1) * PT) for t in SHARDS[sh]])
        y[b, qidx, :] = res.results[c]["out"].T
    return y


# revision 49
# speedup vs baseline: 1.0229x; 1.0229x over previous
"""Distributed GQA attention (RoPE, additive mask) for one TRN2 chip (8 NeuronCores).

Reference semantics (B=4, S=2048, HID=576, 9 q-heads / 3 kv-heads, HD=64):
    q = (x @ wq.T) -> rope;  k = (x @ wk.T) -> rope;  v = x @ wv.T
    scores = q k^T / sqrt(HD) + mask;  attn = softmax(scores);  out = (attn v) @ wo.T

Sharding: 8 cores = 4 batches x 2 query-shards. Each query shard owns 8 of the
16 query tiles (128 tokens each), paired {i, 15-i} alternately so causal work
is balanced. Each core computes all 9 heads for its queries (k/v projections
over the full sequence are duplicated between the 2 cores of a batch --
cheaper than any collective). No cross-core communication.

Device layouts are transposed ([feature, token]) so the TensorEngine contracts
over the head dim; the host pre-transposes inputs, gathers RoPE tables at
position_ids, folds 1/sqrt(HD) into wq, and casts to bf16. Softmax skips the
max-subtraction (scores are O(1) by construction); the denominator comes from
an appended ones-column in V. Masking multiplies exp(scores) by exp(mask);
when the mask is exactly causal this reduces to skipping upper-triangle tiles
plus one per-core 128x128 block multiply per k-tile (SPMD runs one graph on
both shards, so the block mask -- triangular / zeros / ones -- is a per-core
input).
"""

import os
import numpy as np
import ml_dtypes

B, S, HID = 4, 2048, 576
NH, NKV, HD = 9, 3, 64
GQ = NH // NKV
KV = NKV * HD          # 192
PT = 128               # token tile (partition dim)
NQT = S // PT          # 16 query tiles per batch
SHARDS = ([0, 2, 4, 6, 9, 11, 13, 15], [1, 3, 5, 7, 8, 10, 12, 14])
NSH = 8                # q-tiles per shard
QL = NSH * PT          # 1024 queries per core
N_CORES = 8
BF16 = ml_dtypes.bfloat16

# hidden-dim partition chunks: 576 = 4*128 + 64
H_CH = [(0, 128), (128, 128), (256, 128), (384, 128), (512, 64)]

LAST_EXEC_NS = None
LAST_RESULTS = None
_NC_CACHE = {}


# ----------------------------------------------------------------- helpers

def _union_count(kt):
    """max over shards of #tiles >= kt (SPMD union coverage)."""
    return max(sum(1 for t in q if t >= kt) for q in SHARDS)


def _sstart(mode, kt):
    """packed-column start of the union suffix for k-tile kt."""
    if mode != "causal":
        return 0
    return (NSH - _union_count(kt)) * PT


def _chunks(a, b, step=512):
    """split [a, b) at multiples of `step`"""
    out = []
    while a < b:
        c = min(b, (a // step + 1) * step)
        out.append((a, c))
        a = c
    return out


def _mask_mode(attention_mask):
    m = attention_mask[:, 0]  # [B, S, S]
    if not np.any(m):
        return "zeros"
    ki = np.arange(S)
    keep = ki[None, :] <= ki[:, None]  # [S(q), S(k)] lower triangle inclusive
    if np.all(m[:, keep] == 0) and np.all(m[:, ~keep] <= -1e4):
        return "causal"
    return "general"


# ----------------------------------------------------------------- host prep

def _dup_cos(c):
    # c: [T, 64] fp32 -> [128, T] bf16, row r -> cos[dim r%64]
    ct = np.ascontiguousarray(c.T)
    return np.concatenate([ct, ct], axis=0).astype(BF16)


def _dup_sin_signed(s):
    # s: [T, 64] fp32 -> [128, T] bf16, row r: -sin[r%64] if r%64<32 else +sin
    st = s.T.copy()
    st[0:32] = -st[0:32]
    return np.concatenate([st, st], axis=0).astype(BF16)


def _prep_inputs(x, cos, sin, position_ids, attention_mask, wq, wk, wv, wo, mode):
    pos = np.asarray(position_ids)
    wqt = np.ascontiguousarray(wq.T / np.sqrt(HD)).astype(BF16)
    wkt = np.ascontiguousarray(wk.T).astype(BF16)
    wvt = np.ascontiguousarray(wv.T).astype(BF16)
    wot = np.ascontiguousarray(wo.T).astype(BF16)
    triu = np.triu(np.ones((PT, PT), np.float32))

    in_maps = []
    for c in range(N_CORES):
        b, sh = divmod(c, 2)
        qtiles = SHARDS[sh]
        qidx = np.concatenate([np.arange(t * PT, (t + 1) * PT) for t in qtiles])
        cosb = cos[pos[b]]  # [S, 64]
        sinb = sin[pos[b]]
        m = {
            "xt": np.ascontiguousarray(x[b].T).astype(BF16),
            "xq": np.ascontiguousarray(x[b][qidx].T).astype(BF16),
            "wqt": wqt, "wkt": wkt, "wvt": wvt, "wot": wot,
            "cosq": _dup_cos(cosb[qidx]), "ssq": _dup_sin_signed(sinb[qidx]),
            "cosk": _dup_cos(cosb), "ssk": _dup_sin_signed(sinb),
        }
        if mode == "causal":
            # per-core block mask for the first union-suffix block of each kt:
            # diagonal (triangular) if this core owns kt at that position,
            # zeros if its tile there is < kt (fully masked), ones otherwise.
            dm = np.ones((NQT, PT, PT), np.float32)
            for kt in range(NQT):
                p = NSH - _union_count(kt)
                t = qtiles[p]
                if t == kt:
                    dm[kt] = triu
                elif t < kt:
                    dm[kt] = 0.0
            m["dmask"] = dm.reshape(NQT * PT, PT).astype(BF16)
        elif mode == "general":
            mb = attention_mask[b, 0][qidx]  # [QL, S] additive mask, our queries
            m["emskT"] = np.exp(np.minimum(mb.T, 30.0)).astype(BF16)  # [S, QL]
        in_maps.append(m)
    return in_maps


# ------------------------------------------------------------- device kernel

def _head_rows(tiles, h):
    """64-row slice of a row-partitioned [R,C] matrix (tiles of <=128 rows)."""
    off = h * HD
    return tiles[off // PT][off % PT:off % PT + HD]


def _build(mode):
    import concourse.mybir as mybir
    from concourse import bacc, tile
    from contextlib import ExitStack

    dt = mybir.dt
    f32, bf = dt.float32, dt.bfloat16
    AF = mybir.ActivationFunctionType

    nc = bacc.Bacc(
        "TRN2", target_bir_lowering=False, debug=False,
        enable_asserts=False, num_devices=N_CORES,
    )

    def din(name, shape, dty=bf):
        return nc.dram_tensor(name, shape, dty, kind="ExternalInput").ap()

    xt_d = din("xt", [HID, S])
    xq_d = din("xq", [HID, QL])
    wqt_d = din("wqt", [HID, HID])
    wkt_d = din("wkt", [HID, KV])
    wvt_d = din("wvt", [HID, KV])
    wot_d = din("wot", [HID, HID])
    cosq_d = din("cosq", [PT, QL])
    ssq_d = din("ssq", [PT, QL])
    cosk_d = din("cosk", [PT, S])
    ssk_d = din("ssk", [PT, S])
    if mode == "causal":
        dmask_d = din("dmask", [NQT * PT, PT])
    elif mode == "general":
        emskT_d = din("emskT", [S, QL])
    out_d = nc.dram_tensor("out", [HID, QL], f32, kind="ExternalOutput").ap()

    with tile.TileContext(nc) as tc, ExitStack() as ctx:
        ctx.enter_context(nc.allow_low_precision(reason="bf16 attention kernel"))
        sb = ctx.enter_context(tc.tile_pool(name="persist", bufs=1))
        rtmp = ctx.enter_context(tc.tile_pool(name="rtmp", bufs=3))
        attnp = ctx.enter_context(tc.tile_pool(name="attnp", bufs=8))
        outp = ctx.enter_context(tc.tile_pool(name="outp", bufs=3))
        psum = ctx.enter_context(tc.tile_pool(name="psum", bufs=1, space="PSUM"))
        # DRAM bounce for the per-head denominator broadcast (partition-step-0
        # APs are rejected for SBUF sources but fine for DRAM sources)
        dramp = ctx.enter_context(tc.tile_pool(name="dramp", bufs=2, space="DRAM"))

        # ---- persistent loads (row-chunked by 128) ----
        def load_rows(dram, rows, cols, tag):
            tiles = []
            off = 0
            i = 0
            while off < rows:
                sz = min(PT, rows - off)
                t = sb.tile([sz, cols], bf, tag=f"{tag}{i}", name=f"{tag}{i}")
                nc.sync.dma_start(out=t[:, :], in_=dram[off:off + sz, :])
                tiles.append(t)
                off += sz
                i += 1
            return tiles

        def load_rows_cols(dram, rows, cols, tag, cstep=512):
            """like load_rows but DMA in column chunks so consumers of early
            columns unblock before the whole tensor lands"""
            tiles = []
            bounds = []
            off = 0
            i = 0
            while off < rows:
                sz = min(PT, rows - off)
                t = sb.tile([sz, cols], bf, tag=f"{tag}{i}", name=f"{tag}{i}")
                tiles.append(t)
                bounds.append((off, sz))
                off += sz
                i += 1
            for c0, c1 in _chunks(0, cols, cstep):
                for t, (off, sz) in zip(tiles, bounds):
                    nc.sync.dma_start(
                        out=t[:, c0:c1], in_=dram[off:off + sz, c0:c1])
            return tiles

        # load order matters: k/v-projection inputs first
        wkt_sb = load_rows(wkt_d, HID, KV, "wkt")
        wvt_sb = load_rows(wvt_d, HID, KV, "wvt")
        cosk_sb = load_rows(cosk_d, PT, S, "cosk")[0]
        ssk_sb = load_rows(ssk_d, PT, S, "ssk")[0]
        xt_sb = load_rows_cols(xt_d, HID, S, "xt")
        wqt_sb = load_rows(wqt_d, HID, HID, "wqt")
        cosq_sb = load_rows(cosq_d, PT, QL, "cosq")[0]
        ssq_sb = load_rows(ssq_d, PT, QL, "ssq")[0]
        xq_sb = load_rows_cols(xq_d, HID, QL, "xq")
        if mode == "causal":
            dmask_sb = load_rows(dmask_d, NQT * PT, PT, "dmask")
        elif mode == "general":
            emskT_sb = load_rows(emskT_d, S, QL, "emskT")
        wot_sb = load_rows(wot_d, HID, HID, "wot")



        # ---- projections + RoPE ----
        def rope(ps, sz, cos_sb, ss_sb, dsts, c0, c1):
            """dsts: per-head [64, *] sbuf tiles for the sz//64 heads in ps.

            q' = q*cos + rot(q)*sin, with rot/sign folded into partner-row
            reads (psum may be partition-shifted vs the sbuf operands).
            """
            w = c1 - c0
            t1 = rtmp.tile([PT, 512], bf, tag="rt1", name="rt1")
            nc.vector.tensor_mul(t1[:sz, :w], ps[:sz, :w], cos_sb[:sz, c0:c1])
            t2 = rtmp.tile([PT, 512], bf, tag="rt2", name="rt2")
            for g in range(sz // 32):
                p = g ^ 1
                nc.vector.tensor_mul(
                    t2[g * 32:(g + 1) * 32, :w],
                    ps[p * 32:p * 32 + 32, :w],
                    ss_sb[g * 32:g * 32 + 32, c0:c1],
                )
            for hh, dst in enumerate(dsts):
                nc.vector.tensor_add(
                    dst[0:HD, c0:c1],
                    t1[hh * HD:(hh + 1) * HD, :w],
                    t2[hh * HD:(hh + 1) * HD, :w],
                )

        qp_sb = [sb.tile([HD, QL], bf, tag=f"qp{h}", name=f"qp{h}")
                 for h in range(NH)]
        kp_sb = [sb.tile([HD, S], bf, tag=f"kp{g}", name=f"kp{g}")
                 for g in range(NKV)]
        vext_sb = [sb.tile([PT, NKV * (HD + 1)], bf, tag=f"vext{kt}",
                           name=f"vext{kt}") for kt in range(NQT)]
        ao_sb = [sb.tile([sz, QL], bf, tag=f"ao{i}", name=f"ao{i}")
                 for i, (_, sz) in enumerate(H_CH)]

        def emit_qproj(i):
            off, sz = H_CH[i]
            for c0, c1 in _chunks(0, QL):
                ps = psum.tile([PT, 512], f32, tag="ps", bufs=6, name="ps")
                for j, (hoff, hsz) in enumerate(H_CH):
                    nc.tensor.matmul(
                        ps[:sz, :c1 - c0],
                        lhsT=wqt_sb[j][:, off:off + sz],
                        rhs=xq_sb[j][:, c0:c1],
                        start=(j == 0), stop=(j == len(H_CH) - 1),
                    )
                rope(ps, sz, cosq_sb, ssq_sb,
                     qp_sb[2 * i:2 * i + sz // HD], c0, c1)

        def emit_kproj(i, c0, c1):
            off, sz = [(0, 128), (128, 64)][i]
            ps = psum.tile([PT, 512], f32, tag="ps", bufs=6, name="ps")
            for j, (hoff, hsz) in enumerate(H_CH):
                nc.tensor.matmul(
                    ps[:sz, :c1 - c0],
                    lhsT=wkt_sb[j][:, off:off + sz],
                    rhs=xt_sb[j][:, c0:c1],
                    start=(j == 0), stop=(j == len(H_CH) - 1),
                )
            rope(ps, sz, cosk_sb, ssk_sb, kp_sb[2 * i:2 * i + sz // HD], c0, c1)

        def emit_vproj(kt):
            # v with appended ones column: vext[kt] is [128, 3*65]
            nc.gpsimd.memset(vext_sb[kt][:, :], 1.0)
            ps = psum.tile([PT, KV], f32, tag="ps", bufs=6, name="ps")
            for j, (hoff, hsz) in enumerate(H_CH):
                nc.tensor.matmul(
                    ps[:, :],
                    lhsT=xt_sb[j][:, kt * PT:(kt + 1) * PT],
                    rhs=wvt_sb[j][:, :],
                    start=(j == 0), stop=(j == len(H_CH) - 1),
                )
            src = ps[:, :].rearrange("p (g c) -> p g c", c=HD)
            dst = vext_sb[kt][:, :].rearrange(
                "p (g c) -> p g c", c=HD + 1)[:, :, 0:HD]
            nc.vector.tensor_copy(dst, src)

        kts = [kt for kt in range(NQT) if _sstart(mode, kt) < QL]
        last_kt_bank = {
            c0: max(kt for kt in kts if _sstart(mode, kt) < c0 + 512)
            for c0 in range(0, QL, 512)
        }

        def emit_head(h):
            g = h // GQ
            q_ap = qp_sb[h]
            k_ap = kp_sb[g]
            num = psum.tile([HD + 1, QL], f32, tag="num", bufs=1, name="num")
            for kt in kts:
                s0 = _sstart(mode, kt)
                at = attnp.tile([PT, QL], bf, tag="attn", name="attn")
                for c0, c1 in _chunks(s0, QL):
                    sc = psum.tile([PT, 512], f32, tag="ps", bufs=6, name="sc")
                    nc.tensor.matmul(
                        sc[:, :c1 - c0],
                        lhsT=k_ap[:, kt * PT:(kt + 1) * PT],
                        rhs=q_ap[:, c0:c1],
                        start=True, stop=True,
                    )
                    nc.scalar.activation(at[:, c0:c1], sc[:, :c1 - c0], AF.Exp)
                vk = vext_sb[kt][:, g * (HD + 1):(g + 1) * (HD + 1)]
                chunks = _chunks(s0, QL)

                def av(c0, c1):
                    nc.tensor.matmul(
                        num[:, c0:c1], lhsT=vk, rhs=at[:, c0:c1],
                        start=(kt == kts[0]),
                        stop=(kt == last_kt_bank[(c0 // 512) * 512]),
                        skip_group_check=True,
                    )

                if mode == "general":
                    nc.vector.tensor_mul(at[:, :], at[:, :], emskT_sb[kt][:, :])
                    for c0, c1 in chunks:
                        av(c0, c1)
                else:
                    # later chunks don't touch the masked block: issue their
                    # AV matmuls before the mask multiply lands
                    for c0, c1 in chunks[1:]:
                        av(c0, c1)
                    if mode == "causal":
                        nc.vector.tensor_mul(
                            at[:, s0:s0 + PT], at[:, s0:s0 + PT],
                            dmask_sb[kt][:, :])
                    av(*chunks[0])
            # copy num to SBUF immediately so the single psum slot frees for
            # the next head; normalize from the copy. The denominator row is
            # copied to a base-partition-0 tile (PSUM sources may shift
            # partitions; SBUF ones cannot) because the custom-DVE
            # reciprocal_approx_fast only works at base partition 0.
            nsb = rtmp.tile([HD, QL], f32, tag="nsb", bufs=2, name="nsb")
            for c0, c1 in _chunks(0, QL):
                nc.vector.tensor_copy(nsb[:, c0:c1], num[0:HD, c0:c1])
            den = rtmp.tile([1, QL], f32, tag="den", bufs=2, name="den")
            nc.vector.tensor_copy(den[:, :], num[HD:HD + 1, :])
            rec = rtmp.tile([1, QL], f32, tag="rec", bufs=2, name="rec")
            nc.vector.reciprocal_approx_fast(rec[:, :], den[:, :])
            rscr = dramp.tile([1, QL], f32, tag="rscr", name="rscr")
            nc.sync.dma_start(out=rscr[:, :], in_=rec[:, :])
            reps = rtmp.tile([HD, QL], f32, tag="reps", bufs=2, name="reps")
            nc.sync.dma_start(
                out=reps[:, :], in_=rscr[0:1, :].broadcast_to([HD, QL]))
            ao = _head_rows(ao_sb, h)
            for c0, c1 in _chunks(0, QL):
                nc.vector.tensor_mul(
                    ao[:, c0:c1], nsb[:, c0:c1], reps[:, c0:c1])

        # ---- output projection: outT[o, t] = sum_f wot[f, o] * ao[f, t] ----
        # j = 0..3 only need heads 0..7, so a few groups can accumulate while
        # head 8 is still in flight; j = 4 (head 8) lands afterwards
        NJ = len(H_CH)

        def emit_oproj_head(i, c0, c1):
            off, sz = H_CH[i]
            ps = psum.tile([PT, 512], f32, tag="ps", bufs=6, name="ps")
            for j in range(NJ - 1):
                nc.tensor.matmul(
                    ps[:sz, :],
                    lhsT=wot_sb[j][:, off:off + sz],
                    rhs=ao_sb[j][:, c0:c1],
                    start=(j == 0), stop=False,
                    skip_group_check=True,
                )
            return ps

        def emit_oproj_tail(i, c0, c1, ps):
            off, sz = H_CH[i]
            nc.tensor.matmul(
                ps[:sz, :],
                lhsT=wot_sb[NJ - 1][:, off:off + sz],
                rhs=ao_sb[NJ - 1][:, c0:c1],
                start=False, stop=True,
                skip_group_check=True,
            )
            ot = outp.tile([PT, 512], f32, tag="ot", name="ot")
            nc.scalar.copy(ot[:sz, :], ps[:sz, :])
            nc.sync.dma_start(out=out_d[off:off + sz, c0:c1], in_=ot[:sz, :])

        ALL_GROUPS = [(i, c0, c1) for i in range(NJ)
                      for c0, c1 in _chunks(0, QL)]

        # ---- emission schedule ----
        # k/v projections first (v matmuls fill the PE while k's RoPE runs on
        # the vector engine); then q chunks just-in-time interleaved with
        # attention heads so later RoPE overlaps earlier heads' PE work.
        # kv-heads 0/1 (tile row 0) first: heads 0..5 need only those; kv-head
        # 2's projection overlaps the early heads' attention
        for n, (c0, c1) in enumerate(_chunks(0, S)):
            emit_kproj(0, c0, c1)
            for kt in range(4 * n, 4 * n + 4):
                emit_vproj(kt)
        emit_qproj(0)
        emit_qproj(1)
        oproj_ps = {}
        for h in range(NH):
            if h == 1:
                emit_kproj(1, 0, 512)
                emit_kproj(1, 512, 1024)
            if h == 2:
                emit_kproj(1, 1024, 1536)
                emit_kproj(1, 1536, 2048)
            if h == NH - 1:
                # j=0..3 accumulation for the first group overlaps head 8
                for g in ALL_GROUPS[:1]:
                    oproj_ps[g] = emit_oproj_head(*g)
            emit_head(h)
            if 2 + h < len(H_CH):
                emit_qproj(2 + h)
        for g in ALL_GROUPS[1:]:
            oproj_ps[g] = emit_oproj_head(*g)

        for g in ALL_GROUPS:
            emit_oproj_tail(*g, oproj_ps[g])

    nc.compile()
    return nc


def _get_nc(mode):
    if mode not in _NC_CACHE:
        _NC_CACHE[mode] = _build(mode)
    return _NC_CACHE[mode]


def kernel(x, cos, sin, position_ids, attention_mask, wq, wk, wv, wo):
    global LAST_EXEC_NS
    from concourse.bass_utils import run_bass_kernel_spmd

    x = np.asarray(x, np.float32)
    cos = np.asarray(cos, np.float32)
    sin = np.asarray(sin, np.float32)
    position_ids = np.asarray(position_ids)
    attention_mask = np.asarray(attention_mask, np.float32)
    mode = _mask_mode(attention_mask)

    in_maps = _prep_inputs(
        x, cos, sin, position_ids, attention_mask, wq, wk, wv, wo, mode)
    nc = _get_nc(mode)

    trace = os.environ.get("KERNEL_TRACE", "1") != "0"
    try:
        res = run_bass_kernel_spmd(
            nc, in_maps, core_ids=list(range(N_CORES)), trace=trace)
    except Exception:
        if not trace:
            raise
        res = run_bass_kernel_spmd(
            nc, in_maps, core_ids=list(range(N_CORES)), trace=False)
    LAST_EXEC_NS = res.exec_time_ns
    globals()["LAST_RESULTS"] = res

    y = np.empty((B, S, NH * HD), np.float32)
    for c in range(N_CORES):
        b, sh = divmod(c, 2)
        qidx = np.concatenate(
            [np.arange(t * PT, (t + 1) * PT) for t in SHARDS[sh]])
        y[b, qidx, :] = res.results[c]["out"].T
    return y
